# revision 28
# baseline (speedup 1.0000x reference)
"""Trainium2 Bass kernel: topk-masked pseudo-diagonal linear layer.

Math:  a = dykstra_topk(alpha);  W[r,c] = a[(r-c)%n] * V[(r-c)%n, c];
       out = x @ W.T,   with n = 8192, x [1024, 8192], V [8192, 8192].

Strategy (8 NeuronCores, SPMD, no collectives):
  - 2D shard: 4-way over out-features r (R=2048/core) x 2-way over batch
    (BB=512/core).  Each core computes out[b0:b0+512, r0:r0+2048].
  - Host does the cheap, layout-bound work: Dykstra projection of alpha
    (0.4 MFLOP) and the band gather B[c, j] = a[d] * V[d, c] with
    d = (r0 + j - c) % n, emitted in bf16.  The device is a pure
    streaming matmul at the bf16 PE rate: out_tile = xT_block^T @
    B_tile, accumulated over 64 c-tiles in 8 PSUM banks with pipelined
    bf16 LDWEIGHTS.
  - DMA: the h0 phase needs ~220 GB/s (B stream + resident-x fill).
    x streams on the scalar HWDGE ring exclusively; the h0 B stream
    alternates sync HWDGE + gpsimd SWDGE; the first c-tiles' B tiles
    are split into 128 KB halves so the first matmul is gated on small
    cold transfers.
  - Phase schedule for drain overlap: h0 = output cols [0,1024) in all
    8 PSUM banks, then two quarter phases (cols [1024,1536), [1536,2048))
    of 4 banks each, so each phase's PSUM->SBUF copies + output stores
    overlap the next phase's matmuls and the final drain is only 4
    tiles.  Copies alternate DVE/ACT; stores alternate sync/scalar.
"""

import math
import numpy as np

# ---- problem constants (hardcoded; must match reference.py) ----
N = 8192
BATCH = 1024
KTOP = math.ceil((1.0 - 0.9) * N * N / N)  # 820
LR = 0.05
ITERS = 50

CFG_FULL = dict(N=N, BB=512, R=2048, TN=512, GATHER_LANES=0)


def dykstra_host(alpha):
    """Euclidean projection of alpha/LR onto {p: 0<=p<=1, sum p = K} via
    the same 50 Dykstra iterations as the reference (f64 accumulate)."""
    x0 = alpha.astype(np.float64) / LR
    n = x0.shape[0]
    v = x0.copy()
    p = np.zeros_like(v)
    q = np.zeros_like(v)
    for _ in range(ITERS):
        t = v + p
        y = t + (KTOP - t.sum()) / n
        p = t - y
        yq = y + q
        v = np.clip(yq, 0.0, 1.0)
        q = yq - v
    return v.astype(np.float32)


def build_nc(cfg=CFG_FULL):
    """Build + compile the single-core SPMD Bass program."""
    import concourse.bass as bass
    import concourse.tile as tile
    from concourse import bacc, mybir

    f32 = mybir.dt.float32
    bf16 = mybir.dt.bfloat16
    i16 = mybir.dt.int16
    Alu = mybir.AluOpType

    n, bb, r_sh, tn = cfg["N"], cfg["BB"], cfg["R"], cfg["TN"]
    glanes = cfg["GATHER_LANES"]
    rhalf = r_sh // 2
    nct = n // 128          # c-tiles
    nbt = bb // 128         # b-tiles
    nrt = rhalf // tn       # r-subtiles per half
    assert nbt * nrt <= 8
    # x pieces, all on the dedicated scalar ring: small first pieces so
    # the first matmuls start early, 1 MB steady pieces after.  Pieces are
    # even so each maps to whole rows of the pair-packed x layout.
    xpieces = (2, 2, 2, 2, 4, 4, 8, 8, 8, 8, 8, 8)
    assert sum(xpieces) == nct and all(p % 2 == 0 for p in xpieces)

    nc = bacc.Bacc(
        "TRN2", target_bir_lowering=False, debug=False, enable_asserts=False,
        num_swdge_queues=1 + glanes,
    )
    # x is pair-packed: row 128*J + p of xtf_in holds xT rows 256J + p and
    # 256J + 128 + p back to back -> every DMA line is 2*bb bytes (2 KB),
    # which roughly doubles per-queue DMA throughput vs 1 KB lines
    xtf_in = nc.dram_tensor(
        "xtf_in", [n // 2, 2 * bb], bf16, kind="ExternalInput"
    ).ap()
    bt_in = nc.dram_tensor("bt_in", [n, r_sh], bf16, kind="ExternalInput").ap()
    out_d = nc.dram_tensor("out_d", [bb, r_sh], bf16, kind="ExternalOutput").ap()

    with tile.TileContext(nc) as tc:
        with (
            tc.tile_pool(name="idx", bufs=1) as idxp,
            tc.tile_pool(name="xt", bufs=1) as xtp,
            tc.tile_pool(name="vt", bufs=12) as vtp,
            tc.tile_pool(name="vq", bufs=16) as vqp,
            tc.tile_pool(name="ps", bufs=8, space=bass.MemorySpace.PSUM) as psp,
            tc.tile_pool(name="st", bufs=8) as stp,
        ):
            # row-index table for dma_gather lanes: idx[p, c] = 16*c + p
            # (row g of a gather lands at table [p=g%16, col=g//16]); clamp
            # to n-1 so the unused partitions p>=16 stay in bounds
            idx_tab = None
            if glanes:
                idx_tab = idxp.tile([128, n // 16], i16)
                nc.gpsimd.iota(
                    idx_tab[:], pattern=[[16, n // 16]], base=0,
                    channel_multiplier=1,
                )
                nc.gpsimd.tensor_scalar(
                    idx_tab[:], idx_tab[:], float(n - 1), None, op0=Alu.min
                )

            lanes = (nc.sync, nc.scalar, nc.gpsimd)
            # greedy byte-balanced assignment of loads over the three DGE
            # rings; enqueue order per ring = trace order = need order
            ring_bytes = [0, 0, 0]

            def pick_lane(nbytes):
                i = min(range(3), key=lambda k: ring_bytes[k])
                ring_bytes[i] += nbytes
                return lanes[i]

            def account(i, nbytes):
                ring_bytes[i] += nbytes

            # resident xT piece tiles (pair-packed free dim: [pair, 2*bb])
            xt_tiles = []
            xt_of_ct = {}
            ct0 = 0
            for xc, cpc in enumerate(xpieces):
                xt_sb = xtp.tile([128, cpc // 2, 2 * bb], bf16, name=f"xt{xc}")
                xt_tiles.append((xt_sb, ct0, cpc))
                for ci in range(cpc):
                    xt_of_ct[ct0 + ci] = (xt_sb, ci // 2, (ci % 2) * bb)
                ct0 += cpc

            def load_xt(xc):
                xt_sb, c0, cpc = xt_tiles[xc]
                w = xc % (1 + glanes) if glanes else 0
                if w == 0:
                    nc.scalar.dma_start(
                        xt_sb[:],
                        xtf_in[64 * c0 : 64 * (c0 + cpc), :].rearrange(
                            "(j p) b -> p j b", p=128
                        ),
                    )
                else:
                    # identity row-gather on SWDGE queue w: adds an extra
                    # DMA queue to the x fill during the h0 crunch window
                    nc.gpsimd.dma_gather(
                        xt_sb[:],
                        xtf_in,
                        idx_tab[:16, 8 * c0 : 8 * (c0 + cpc)],
                        num_idxs=128 * cpc,
                        num_idxs_reg=128 * cpc,
                        elem_size=bb,
                        queue_num=w,
                    )

            def copy_store(i, ps_t, bt, col0, eng=None):
                """PSUM -> SBUF (f32->bf16) on DVE/ACT (/gpsimd), then
                store on alternating sync/scalar HWDGE rings."""
                st_t = stp.tile([128, tn], bf16, tag="st")
                eng = i % 2 if eng is None else eng
                if eng == 0:
                    nc.vector.tensor_scalar(
                        st_t[:], ps_t[:], 0.0, None, op0=Alu.add
                    )
                elif eng == 2:
                    nc.gpsimd.tensor_scalar(
                        st_t[:], ps_t[:], 0.0, None, op0=Alu.add
                    )
                else:
                    nc.scalar.copy(st_t[:], ps_t[:])
                # stores on the two hardware-DGE rings only (a store queued
                # on the gpsimd SWDGE ring stalls final drain)
                ring = 0 if i % 2 == 0 else 1
                account(ring, 128 * tn * 2)
                lanes[ring].dma_start(
                    out_d[128 * bt : 128 * (bt + 1), col0 : col0 + tn],
                    st_t[:],
                )

            vt_tiles = {}

            def load_vt(si):
                vt_t = vtp.tile([128, 1, rhalf], bf16, tag="vt")
                vt_tiles[si] = vt_t
                lanes[(0, 2)[si % 2]].dma_start(
                    vt_t[:],
                    bt_in[128 * si : 128 * (si + 1), :rhalf].rearrange(
                        "(ct p) b -> p ct b", p=128
                    ),
                )

            # ---- prefetch ramp: the first c-tiles' B tiles split into
            # half-width pieces so the first matmuls are gated on 128 KB
            # transfers; x pieces interleaved in need order ----
            RAMP_CT = 4
            xpc_start = {}
            c0 = 0
            for xc, cpc in enumerate(xpieces):
                xpc_start[c0] = xc
                c0 += cpc
            vt_halves = {}

            def load_vt_half(ct, half):
                vt_h = vqp.tile([128, 1, tn], bf16, tag="vq")
                vt_halves[(ct, half)] = vt_h
                lanes[(0, 2)[(2 * ct + half) % 2]].dma_start(
                    vt_h[:],
                    bt_in[128 * ct : 128 * (ct + 1),
                          tn * half : tn * (half + 1)].rearrange(
                        "(ct p) b -> p ct b", p=128
                    ),
                )

            # ramp halves lead the B rings; the full x fill queues on
            # scalar upfront and streams continuously behind piece 0
            for ct in range(RAMP_CT):
                load_vt_half(ct, 0)
                load_vt_half(ct, 1)
            for xc in range(len(xpieces)):
                load_xt(xc)

            # ---- phase h0: output cols [0, rhalf), 8 PSUM banks ----
            ps_tiles = [
                psp.tile([128, tn], f32, tag="mm", name=f"ps_0_{i}")
                for i in range(nbt * nrt)
            ]
            for ct in range(nct):
                if ct >= RAMP_CT and ct not in vt_tiles:
                    load_vt(ct)
                vt_t = None if ct < RAMP_CT else vt_tiles.pop(ct)
                xt_sb, xi, xo = xt_of_ct[ct]
                for bt in range(nbt):
                    lhsT = xt_sb[:, xi, xo + 128 * bt : xo + 128 * (bt + 1)]
                    for rt in range(nrt):
                        rhs = (
                            vt_halves[(ct, rt)][:, 0, :]
                            if ct < RAMP_CT
                            else vt_t[:, 0, tn * rt : tn * (rt + 1)]
                        )
                        nc.tensor.matmul(
                            ps_tiles[bt * nrt + rt][:],
                            lhsT,
                            rhs,
                            start=(ct == 0),
                            stop=(ct == nct - 1),
                        )
            for bt in range(nbt):
                for rt in range(nrt):
                    i = bt * nrt + rt
                    copy_store(i, ps_tiles[i], bt, tn * rt)

            # ---- phases q0/q1: output cols rhalf+[0,tn) and rhalf+[tn,2tn),
            # 4 PSUM banks each so the drain of one quarter overlaps the
            # next quarter's matmuls (and the final drain is half-size) ----
            for q in range(rhalf // tn):
                col0 = rhalf + tn * q
                ps_q = [
                    psp.tile([128, tn], f32, tag="mm", name=f"ps_1{q}_{i}")
                    for i in range(nbt)
                ]
                vq_tiles = {}

                def load_vq(si, q=q, vq_tiles=vq_tiles, col0=col0):
                    vq_t = vqp.tile([128, 1, tn], bf16, tag="vq")
                    vq_tiles[si] = vq_t
                    lanes[(0, 2, 1)[si % 3]].dma_start(
                        vq_t[:],
                        bt_in[128 * si : 128 * (si + 1), col0 : col0 + tn]
                        .rearrange("(ct p) b -> p ct b", p=128),
                    )

                load_vq(0)
                load_vq(1)
                for ct in range(nct):
                    if ct not in vq_tiles:
                        load_vq(ct)
                    vq_t = vq_tiles.pop(ct)
                    xt_sb, xi, xo = xt_of_ct[ct]
                    for bt in range(nbt):
                        nc.tensor.matmul(
                            ps_q[bt][:],
                            xt_sb[:, xi, xo + 128 * bt : xo + 128 * (bt + 1)],
                            vq_t[:, 0, :],
                            start=(ct == 0),
                            stop=(ct == nct - 1),
                        )
                for bt in range(nbt):
                    copy_store(bt, ps_q[bt], bt, col0)
    nc.compile()
    return nc


# ---------------- host-side prep / gather ----------------

def host_prep(x, V, alpha, cfg=CFG_FULL):
    """Build the 8 per-core input maps. Core id = ib*4 + ir."""
    import ml_dtypes

    n, bb, r_sh = cfg["N"], cfg["BB"], cfg["R"]
    x = np.ascontiguousarray(x, dtype=np.float32)
    V = np.ascontiguousarray(V, dtype=np.float32)
    alpha = np.ascontiguousarray(alpha, dtype=np.float32)

    a = dykstra_host(alpha)

    # AT[c, d] = a[d] * V[d, c]; band row c of the sheared gather is the
    # contiguous run AT3[c, n - c : n - c + n + r_sh] (zero-copy strided view)
    AT = np.ascontiguousarray(V.T) * a[None, :]
    AT3 = np.concatenate([AT, AT, AT[:, :r_sh]], axis=1)
    AT3 = np.ascontiguousarray(AT3)
    pitch = AT3.strides[0]
    isz = AT3.itemsize
    Bview = np.lib.stride_tricks.as_strided(
        AT3[:, n:], shape=(n, n + r_sh), strides=(pitch - isz, isz)
    )
    # Bview[c, m] = AT3[c, n - c + m] = a[(m - c) % n] * V[(m - c) % n, c]
    bts = [
        np.ascontiguousarray(Bview[:, r0 : r0 + r_sh].astype(ml_dtypes.bfloat16))
        for r0 in range(0, n, r_sh)
    ]
    del AT, AT3, Bview

    xb = x.astype(ml_dtypes.bfloat16)
    # pair-pack: row 128*J + p holds xT rows 256J + p and 256J + 128 + p
    xtfs = []
    for b0 in range(0, x.shape[0], bb):
        xt = np.ascontiguousarray(xb[b0 : b0 + bb].T)          # [n, bb]
        xt2 = xt.reshape(n // 256, 2, 128, bb).transpose(0, 2, 1, 3)
        xtfs.append(np.ascontiguousarray(xt2.reshape(n // 2, 2 * bb)))

    in_maps = []
    for ib in range(x.shape[0] // bb):
        for ir in range(n // r_sh):
            in_maps.append({"xtf_in": xtfs[ib], "bt_in": bts[ir]})
    return in_maps


_nc_cache = None


def kernel(x, V, alpha):
    """Full-input, full-output entry point. Shards over 8 NeuronCores."""
    from concourse import bass_utils

    global _nc_cache
    if _nc_cache is None:
        _nc_cache = build_nc(CFG_FULL)
    nc = _nc_cache

    in_maps = host_prep(x, V, alpha, CFG_FULL)
    res = bass_utils.run_bass_kernel_spmd(nc, in_maps, core_ids=list(range(8)))
    kernel.last_results = res

    bb, r_sh = CFG_FULL["BB"], CFG_FULL["R"]
    out = np.empty((BATCH, N), np.float32)
    for core, rmap in enumerate(res.results):
        ib, ir = divmod(core, N // r_sh)
        out[bb * ib : bb * (ib + 1), r_sh * ir : r_sh * (ir + 1)] = np.asarray(
            rmap["out_d"]
        ).astype(np.float32)
    return out



# revision 36
# speedup vs baseline: 1.0303x; 1.0303x over previous
"""Trainium2 Bass kernel: topk-masked pseudo-diagonal linear layer.

Math:  a = dykstra_topk(alpha);  W[r,c] = a[(r-c)%n] * V[(r-c)%n, c];
       out = x @ W.T,   with n = 8192, x [1024, 8192], V [8192, 8192].

Strategy (8 NeuronCores, SPMD, no collectives):
  - 2D shard: 4-way over out-features r (R=2048/core) x 2-way over batch
    (BB=512/core).  Each core computes out[b0:b0+512, r0:r0+2048].
  - Host does the cheap, layout-bound work: Dykstra projection of alpha
    (0.4 MFLOP) and the band gather B[c, j] = a[d] * V[d, c] with
    d = (r0 + j - c) % n, emitted in bf16.  The device is a pure
    streaming matmul at the bf16 PE rate: out_tile = xT_block^T @
    B_tile, accumulated over 64 c-tiles in 8 PSUM banks with pipelined
    bf16 LDWEIGHTS.
  - DMA: the h0 phase needs ~220 GB/s (B stream + resident-x fill).
    x is pair-packed in DRAM (two xT rows per 2 KB line, ~2x per-queue
    throughput vs 1 KB lines) and streams on the scalar HWDGE ring
    EXCLUSIVELY -- mixing any B traffic into the scalar ring serializes
    its DMA-semaphore pool against the 1 MB x pieces and stalls the PE
    mid-fill.  The B stream alternates sync HWDGE + gpsimd SWDGE; the
    first c-tiles' B tiles are split into 128 KB halves (own tile pool:
    a pool shallower than the ramp-half count blocks the ring engines)
    so the first matmul is gated on small cold transfers.
  - Phase schedule for drain overlap: h0 = output cols [0,1024) in all
    8 PSUM banks, then two quarter phases (cols [1024,1536), [1536,2048))
    of 4 banks each, so each phase's PSUM->SBUF copies + output stores
    overlap the next phase's matmuls and the final drain is only 4
    tiles.  Copies alternate DVE/ACT; stores alternate sync/scalar.
"""

import math
import numpy as np

# ---- problem constants (hardcoded; must match reference.py) ----
N = 8192
BATCH = 1024
KTOP = math.ceil((1.0 - 0.9) * N * N / N)  # 820
LR = 0.05
ITERS = 50

CFG_FULL = dict(N=N, BB=512, R=2048, TN=512, GATHER_LANES=0)


def dykstra_host(alpha):
    """Euclidean projection of alpha/LR onto {p: 0<=p<=1, sum p = K} via
    the same 50 Dykstra iterations as the reference (f64 accumulate)."""
    x0 = alpha.astype(np.float64) / LR
    n = x0.shape[0]
    v = x0.copy()
    p = np.zeros_like(v)
    q = np.zeros_like(v)
    for _ in range(ITERS):
        t = v + p
        y = t + (KTOP - t.sum()) / n
        p = t - y
        yq = y + q
        v = np.clip(yq, 0.0, 1.0)
        q = yq - v
    return v.astype(np.float32)


def build_nc(cfg=CFG_FULL):
    """Build + compile the single-core SPMD Bass program."""
    import concourse.bass as bass
    import concourse.tile as tile
    from concourse import bacc, mybir

    f32 = mybir.dt.float32
    bf16 = mybir.dt.bfloat16
    i16 = mybir.dt.int16
    Alu = mybir.AluOpType

    n, bb, r_sh, tn = cfg["N"], cfg["BB"], cfg["R"], cfg["TN"]
    glanes = cfg["GATHER_LANES"]
    rhalf = r_sh // 2
    nct = n // 128          # c-tiles
    nbt = bb // 128         # b-tiles
    nrt = rhalf // tn       # r-subtiles per half
    assert nbt * nrt <= 8
    # x pieces, all on the dedicated scalar ring: small first pieces so
    # the first matmuls start early, 1 MB steady pieces after.  Pieces are
    # even so each maps to whole rows of the pair-packed x layout.
    xpieces = (2, 2, 2, 2, 4, 4, 8, 8, 8, 8, 8, 8)
    assert sum(xpieces) == nct and all(p % 2 == 0 for p in xpieces)

    nc = bacc.Bacc(
        "TRN2", target_bir_lowering=False, debug=False, enable_asserts=False,
        num_swdge_queues=1 + glanes,
    )
    # x is pair-packed: row 128*J + p of xtf_in holds xT rows 256J + p and
    # 256J + 128 + p back to back -> every DMA line is 2*bb bytes (2 KB),
    # which roughly doubles per-queue DMA throughput vs 1 KB lines
    xtf_in = nc.dram_tensor(
        "xtf_in", [n // 2, 2 * bb], bf16, kind="ExternalInput"
    ).ap()
    bt_in = nc.dram_tensor("bt_in", [n, r_sh], bf16, kind="ExternalInput").ap()
    out_d = nc.dram_tensor("out_d", [bb, r_sh], bf16, kind="ExternalOutput").ap()

    with tile.TileContext(nc) as tc:
        with (
            tc.tile_pool(name="idx", bufs=1) as idxp,
            tc.tile_pool(name="xt", bufs=1) as xtp,
            tc.tile_pool(name="vt", bufs=12) as vtp,
            tc.tile_pool(name="vq", bufs=8) as vqp,
            tc.tile_pool(name="rp", bufs=8) as rpp,
            tc.tile_pool(name="ps", bufs=8, space=bass.MemorySpace.PSUM) as psp,
            tc.tile_pool(name="st", bufs=8) as stp,
        ):
            # row-index table for dma_gather lanes: idx[p, c] = 16*c + p
            # (row g of a gather lands at table [p=g%16, col=g//16]); clamp
            # to n-1 so the unused partitions p>=16 stay in bounds
            idx_tab = None
            if glanes:
                idx_tab = idxp.tile([128, n // 16], i16)
                nc.gpsimd.iota(
                    idx_tab[:], pattern=[[16, n // 16]], base=0,
                    channel_multiplier=1,
                )
                nc.gpsimd.tensor_scalar(
                    idx_tab[:], idx_tab[:], float(n - 1), None, op0=Alu.min
                )

            lanes = (nc.sync, nc.scalar, nc.gpsimd)
            # greedy byte-balanced assignment of loads over the three DGE
            # rings; enqueue order per ring = trace order = need order
            ring_bytes = [0, 0, 0]

            def pick_lane(nbytes):
                i = min(range(3), key=lambda k: ring_bytes[k])
                ring_bytes[i] += nbytes
                return lanes[i]

            def account(i, nbytes):
                ring_bytes[i] += nbytes

            # resident xT piece tiles (pair-packed free dim: [pair, 2*bb])
            xt_tiles = []
            xt_of_ct = {}
            ct0 = 0
            for xc, cpc in enumerate(xpieces):
                xt_sb = xtp.tile([128, cpc // 2, 2 * bb], bf16, name=f"xt{xc}")
                xt_tiles.append((xt_sb, ct0, cpc))
                for ci in range(cpc):
                    xt_of_ct[ct0 + ci] = (xt_sb, ci // 2, (ci % 2) * bb)
                ct0 += cpc

            def load_xt(xc):
                xt_sb, c0, cpc = xt_tiles[xc]
                w = xc % (1 + glanes) if glanes else 0
                if w == 0:
                    if xc == 0:
                        # first piece split in two so the very first matmul
                        # is gated on a 128 KB transfer, not 256 KB
                        for h2 in range(2):
                            nc.scalar.dma_start(
                                xt_sb[:, 0 : cpc // 2, bb * h2 : bb * (h2 + 1)],
                                xtf_in[64 * c0 : 64 * (c0 + cpc),
                                       bb * h2 : bb * (h2 + 1)].rearrange(
                                    "(j p) b -> p j b", p=128
                                ),
                            )
                    else:
                        nc.scalar.dma_start(
                            xt_sb[:],
                            xtf_in[64 * c0 : 64 * (c0 + cpc), :].rearrange(
                                "(j p) b -> p j b", p=128
                            ),
                        )
                else:
                    # identity row-gather on SWDGE queue w: adds an extra
                    # DMA queue to the x fill during the h0 crunch window
                    nc.gpsimd.dma_gather(
                        xt_sb[:],
                        xtf_in,
                        idx_tab[:16, 8 * c0 : 8 * (c0 + cpc)],
                        num_idxs=128 * cpc,
                        num_idxs_reg=128 * cpc,
                        elem_size=bb,
                        queue_num=w,
                    )

            def copy_store(i, ps_t, bt, col0, eng=None):
                """PSUM -> SBUF (f32->bf16) on DVE/ACT (/gpsimd), then
                store on alternating sync/scalar HWDGE rings."""
                st_t = stp.tile([128, tn], bf16, tag="st")
                eng = i % 2 if eng is None else eng
                if eng == 0:
                    nc.vector.tensor_scalar(
                        st_t[:], ps_t[:], 0.0, None, op0=Alu.add
                    )
                elif eng == 2:
                    nc.gpsimd.tensor_scalar(
                        st_t[:], ps_t[:], 0.0, None, op0=Alu.add
                    )
                else:
                    nc.scalar.copy(st_t[:], ps_t[:])
                # stores on the two hardware-DGE rings only (a store queued
                # on the gpsimd SWDGE ring stalls final drain)
                ring = 0 if i % 2 == 0 else 1
                account(ring, 128 * tn * 2)
                lanes[ring].dma_start(
                    out_d[128 * bt : 128 * (bt + 1), col0 : col0 + tn],
                    st_t[:],
                )

            vt_tiles = {}

            def load_vt(si):
                vt_t = vtp.tile([128, 1, rhalf], bf16, tag="vt")
                vt_tiles[si] = vt_t
                lanes[(0, 2)[si % 2]].dma_start(
                    vt_t[:],
                    bt_in[128 * si : 128 * (si + 1), :rhalf].rearrange(
                        "(ct p) b -> p ct b", p=128
                    ),
                )

            # ---- prefetch ramp: the first c-tiles' B tiles split into
            # half-width pieces so the first matmuls are gated on 128 KB
            # transfers; x pieces interleaved in need order ----
            RAMP_CT = 4
            xpc_start = {}
            c0 = 0
            for xc, cpc in enumerate(xpieces):
                xpc_start[c0] = xc
                c0 += cpc
            vt_halves = {}

            def load_vt_half(ct, half):
                vt_h = rpp.tile([128, 1, tn], bf16, tag="rp")
                vt_halves[(ct, half)] = vt_h
                if ct == 0:
                    # quarter-split across both B rings: the first matmul's
                    # rhs is gated on two parallel 64 KB cold transfers
                    for q2 in range(2):
                        lanes[(0, 2)[(half + q2) % 2]].dma_start(
                            vt_h[:, 0, 256 * q2 : 256 * (q2 + 1)],
                            bt_in[0:128,
                                  tn * half + 256 * q2 :
                                  tn * half + 256 * (q2 + 1)].rearrange(
                                "(ct p) b -> p (ct b)", p=128
                            ),
                        )
                else:
                    lanes[(0, 2)[(2 * ct + half) % 2]].dma_start(
                        vt_h[:],
                        bt_in[128 * ct : 128 * (ct + 1),
                              tn * half : tn * (half + 1)].rearrange(
                            "(ct p) b -> p ct b", p=128
                        ),
                    )

            # ramp halves lead the B rings; the full x fill queues on
            # scalar upfront and streams continuously behind piece 0
            for ct in range(RAMP_CT):
                load_vt_half(ct, 0)
                load_vt_half(ct, 1)
            for xc in range(len(xpieces)):
                load_xt(xc)

            # ---- phase h0: output cols [0, rhalf), 8 PSUM banks ----
            ps_tiles = [
                psp.tile([128, tn], f32, tag="mm", name=f"ps_0_{i}")
                for i in range(nbt * nrt)
            ]
            for ct in range(nct):
                if ct >= RAMP_CT and ct not in vt_tiles:
                    load_vt(ct)
                vt_t = None if ct < RAMP_CT else vt_tiles.pop(ct)
                xt_sb, xi, xo = xt_of_ct[ct]
                for bt in range(nbt):
                    lhsT = xt_sb[:, xi, xo + 128 * bt : xo + 128 * (bt + 1)]
                    for rt in range(nrt):
                        rhs = (
                            vt_halves[(ct, rt)][:, 0, :]
                            if ct < RAMP_CT
                            else vt_t[:, 0, tn * rt : tn * (rt + 1)]
                        )
                        nc.tensor.matmul(
                            ps_tiles[bt * nrt + rt][:],
                            lhsT,
                            rhs,
                            start=(ct == 0),
                            stop=(ct == nct - 1),
                        )
            for bt in range(nbt):
                for rt in range(nrt):
                    i = bt * nrt + rt
                    copy_store(i, ps_tiles[i], bt, tn * rt)

            # ---- phases q0/q1: output cols rhalf+[0,tn) and rhalf+[tn,2tn),
            # 4 PSUM banks each so the drain of one quarter overlaps the
            # next quarter's matmuls (and the final drain is half-size) ----
            for q in range(rhalf // tn):
                col0 = rhalf + tn * q
                ps_q = [
                    psp.tile([128, tn], f32, tag="mm", name=f"ps_1{q}_{i}")
                    for i in range(nbt)
                ]
                vq_tiles = {}

                def load_vq(si, q=q, vq_tiles=vq_tiles, col0=col0):
                    vq_t = vqp.tile([128, 1, tn], bf16, tag="vq")
                    vq_tiles[si] = vq_t
                    # sync+gpsimd only: the scalar ring must stay exclusive
                    # to the x fill, or premature vq prefetch serializes the
                    # scalar DMA-sem pool against the big x pieces
                    lanes[(0, 2)[si % 2]].dma_start(
                        vq_t[:],
                        bt_in[128 * si : 128 * (si + 1), col0 : col0 + tn]
                        .rearrange("(ct p) b -> p ct b", p=128),
                    )

                load_vq(0)
                load_vq(1)
                for ct in range(nct):
                    if ct not in vq_tiles:
                        load_vq(ct)
                    vq_t = vq_tiles.pop(ct)
                    xt_sb, xi, xo = xt_of_ct[ct]
                    for bt in range(nbt):
                        nc.tensor.matmul(
                            ps_q[bt][:],
                            xt_sb[:, xi, xo + 128 * bt : xo + 128 * (bt + 1)],
                            vq_t[:, 0, :],
                            start=(ct == 0),
                            stop=(ct == nct - 1),
                        )
                for bt in range(nbt):
                    copy_store(bt, ps_q[bt], bt, col0)
    nc.compile()
    return nc


# ---------------- host-side prep / gather ----------------

def host_prep(x, V, alpha, cfg=CFG_FULL):
    """Build the 8 per-core input maps. Core id = ib*4 + ir."""
    import ml_dtypes

    n, bb, r_sh = cfg["N"], cfg["BB"], cfg["R"]
    x = np.ascontiguousarray(x, dtype=np.float32)
    V = np.ascontiguousarray(V, dtype=np.float32)
    alpha = np.ascontiguousarray(alpha, dtype=np.float32)

    a = dykstra_host(alpha)

    # AT[c, d] = a[d] * V[d, c]; band row c of the sheared gather is the
    # contiguous run AT3[c, n - c : n - c + n + r_sh] (zero-copy strided view)
    AT = np.ascontiguousarray(V.T) * a[None, :]
    AT3 = np.concatenate([AT, AT, AT[:, :r_sh]], axis=1)
    AT3 = np.ascontiguousarray(AT3)
    pitch = AT3.strides[0]
    isz = AT3.itemsize
    Bview = np.lib.stride_tricks.as_strided(
        AT3[:, n:], shape=(n, n + r_sh), strides=(pitch - isz, isz)
    )
    # Bview[c, m] = AT3[c, n - c + m] = a[(m - c) % n] * V[(m - c) % n, c]
    bts = [
        np.ascontiguousarray(Bview[:, r0 : r0 + r_sh].astype(ml_dtypes.bfloat16))
        for r0 in range(0, n, r_sh)
    ]
    del AT, AT3, Bview

    xb = x.astype(ml_dtypes.bfloat16)
    # pair-pack: row 128*J + p holds xT rows 256J + p and 256J + 128 + p
    xtfs = []
    for b0 in range(0, x.shape[0], bb):
        xt = np.ascontiguousarray(xb[b0 : b0 + bb].T)          # [n, bb]
        xt2 = xt.reshape(n // 256, 2, 128, bb).transpose(0, 2, 1, 3)
        xtfs.append(np.ascontiguousarray(xt2.reshape(n // 2, 2 * bb)))

    in_maps = []
    for ib in range(x.shape[0] // bb):
        for ir in range(n // r_sh):
            in_maps.append({"xtf_in": xtfs[ib], "bt_in": bts[ir]})
    return in_maps


_nc_cache = None


def kernel(x, V, alpha):
    """Full-input, full-output entry point. Shards over 8 NeuronCores."""
    from concourse import bass_utils

    global _nc_cache
    if _nc_cache is None:
        _nc_cache = build_nc(CFG_FULL)
    nc = _nc_cache

    in_maps = host_prep(x, V, alpha, CFG_FULL)
    res = bass_utils.run_bass_kernel_spmd(nc, in_maps, core_ids=list(range(8)))
    kernel.last_results = res

    bb, r_sh = CFG_FULL["BB"], CFG_FULL["R"]
    out = np.empty((BATCH, N), np.float32)
    for core, rmap in enumerate(res.results):
        ib, ir = divmod(core, N // r_sh)
        out[bb * ib : bb * (ib + 1), r_sh * ir : r_sh * (ir + 1)] = np.asarray(
            rmap["out_d"]
        ).astype(np.float32)
    return out

